# revision 1
# baseline (speedup 1.0000x reference)
"""Trainium2 Bass kernel for nn_LossTDSurv — v2.4 (prefix-truncated, bf16,
merged multiply-trees, class-contiguous packing).

 - Only the prefix h[0..idx] of each row is used by the loss; host ships
   just that, width-binned to multiples of 8 and packed CONTIGUOUSLY per
   width-class (rows of the 8 idx-groups of a class share slots; the
   per-slot-element constant 2^-(v-1) table makes mixed-v slots fine).
   Row padding is only the class tail (<1%).
 - Transport is 2*(1-h) in bf16; cond_sum A = ln(prod) via pairwise
   multiply TREES on the vector engine (tensor_tensor bf16 = 2
   elem/lane/cycle).  The factor 2 keeps P' = 2^(v-1)*P inside the ACT
   Ln spline's accurate range (HW Ln saturates ~-45.75 below ~1e-20).
   Corrections: sum(A) via per-core host constants; sum(e*A) via
   ln2*sum(e*max(idx-1,0)); sum(e) is an event count — all derived from
   target metadata during packing.
 - P = P'*2^-(v-1) is exact in bf16; log_Wt = ln(1e-8 + (1-P)) needs
   neither Exp nor a clamp op.
 - Classes ordered [8,16,32,64 | 24,48 | 40,56]: pow2 trees merge into a
   shared 8->4->2->1 tail, (24,48) share 12->6->3; the epilogue runs in
   two halves, the first overlapping the later trees.
"""

import numpy as np
import ml_dtypes

BF16 = ml_dtypes.bfloat16
LN2 = float(np.log(2.0))

B_TOTAL = 524288
T = 64
N_CORES = 8
G = 64

# classes in processing order: (width, v_first, v_last)
CLASSES = [
    (8, 2, 9), (16, 10, 17), (32, 26, 33), (64, 58, 63),
    (24, 18, 25), (48, 42, 49), (40, 34, 41), (56, 50, 57),
]

def _chunks(plan):
    """A-region chunks: {index: (col_lo, col_hi)}."""
    aoff = plan["aoff"]
    return {
        0: (aoff[0], aoff[1]),                   # c8
        1: (aoff[1], aoff[2]),                   # c16
        2: (aoff[2], aoff[3]),                   # c32
        34: (aoff[3], aoff[4]),                  # c64
        5: (aoff[4], aoff[6]),                   # c24+c48
        67: (aoff[6], aoff[8]),                  # c40+c56
    }


_CACHE = {}


def _plan(all_counts):
    """all_counts: [n_cores, G] rows per (core, idx value).  Returns the
    layout plan (slot columns per class etc.) shared by all cores."""
    S01 = int(max(-(-int(c[0] + c[1]) // 128) for c in all_counts))
    S = []
    for (w, v0, v1) in CLASSES:
        n = max(int(c[v0:v1 + 1].sum()) for c in all_counts)
        S.append(-(-n // 128))
    pcol = [S01]
    for s in S:
        pcol.append(pcol[-1] + s)
    P = pcol[-1]
    aoff = [0]
    for ci, (w, v0, v1) in enumerate(CLASSES):
        aoff.append(aoff[-1] + S[ci] * w)
    AW = aoff[-1]
    return dict(S01=S01, S=S, pcol=pcol, P=P, aoff=aoff, AW=AW)


def _build_nc(plan):
    import concourse.bacc as bacc
    import concourse.mybir as mybir
    import concourse.tile as tile

    f32 = mybir.dt.float32
    bf16 = mybir.dt.bfloat16
    AF = mybir.ActivationFunctionType
    OP = mybir.AluOpType
    AX = mybir.AxisListType

    S01, S, pcol, P, aoff, AW = (plan[k] for k in
                                 ("S01", "S", "pcol", "P", "aoff", "AW"))

    nc = bacc.Bacc("TRN2", target_bir_lowering=False, debug=False)

    chunks = _chunks(plan)
    ab_d = {
        i: nc.dram_tensor(f"ab{i}", [128, c1 - c0], bf16,
                          kind="ExternalInput")
        for i, (c0, c1) in chunks.items()
    }
    bhe_d = nc.dram_tensor("bhe", [128, 5 * P], bf16, kind="ExternalInput")
    c2_d = nc.dram_tensor("c2", [128, P], bf16, kind="ExternalInput")
    part_d = nc.dram_tensor("partials", [128, 12], f32, kind="ExternalOutput")

    with tile.TileContext(nc) as tc:
        with tc.tile_pool(name="pers", bufs=1) as pers:
            A = pers.tile([128, AW], bf16, tag="A")
            BHE = pers.tile([128, 5 * P], bf16, tag="BHE")
            C2t = pers.tile([128, P], bf16, tag="C2t")
            Pb = pers.tile([128, P], bf16, tag="Pb")
            Pt = pers.tile([128, P], bf16, tag="Pt")
            s0t = pers.tile([128, 2048], bf16, tag="s0")
            s1t = pers.tile([128, 1024], bf16, tag="s1")
            w8t = pers.tile([128, (S[0] + S[1] + S[2] + S[3]) * 8], bf16,
                            tag="w8")
            w12t = pers.tile([128, (S[4] + S[5]) * 12], bf16, tag="w12")
            Ab = pers.tile([128, P], f32, tag="Ab")
            Lb = pers.tile([128, 2 * P], bf16, tag="Lb")
            loghv = pers.tile([128, P], bf16, tag="loghv")
            wt = pers.tile([128, P], bf16, tag="wt")
            logwt = pers.tile([128, P], bf16, tag="logwt")
            sct = pers.tile([128, P], bf16, tag="sct")
            acc = pers.tile([128, 12], f32, tag="acc")

            Eb = BHE[:, 4 * P:5 * P]

            nc.gpsimd.memset(Pb[:, 0:S01], 1.0)   # v=0,1: empty prefix
            nc.gpsimd.memset(acc[:, 9:12], 0.0)
            # warmup Ln so the ACT table load overlaps the first DMA
            nc.scalar.activation(sct[:, 0:2], Pb[:, 0:2], AF.Ln)

            def dma_chunk(i, eng):
                a, b = chunks[i]
                eng.dma_start(A[:, a:b], ab_d[i][:])

            # two HWDGE rings, chunk order = tree need order; class-8
            # lands directly in the shared width-8 buffer; c64 is split
            # across both rings
            nc.sync.dma_start(w8t[:, 0:S[0] * 8], ab_d[0][:])
            dma_chunk(1, nc.sync)        # c16
            dma_chunk(2, nc.scalar)      # c32
            dma_chunk(34, nc.sync)       # c64
            nc.scalar.dma_start(BHE[:], bhe_d[:])
            dma_chunk(5, nc.sync)        # c24+c48
            nc.scalar.dma_start(C2t[:], c2_d[:])
            dma_chunk(67, nc.sync)       # c40+c56

            def aview(ci):
                w = CLASSES[ci][0]
                return A[:, aoff[ci]:aoff[ci] + S[ci] * w].rearrange(
                    "p (s w) -> p s w", w=w)

            def halve(src, out, W):
                nc.vector.tensor_tensor(
                    out=out, in0=src[:, :, :W // 2], in1=src[:, :, W // 2:],
                    op=OP.mult)

            def view(tile_, cols, Sn, W):
                return tile_[:, cols:cols + Sn * W].rearrange(
                    "p (s w) -> p s w", w=W)

            # ---- pow2 classes feed shared width-8 buffer ----
            Sp = S[0] + S[1] + S[2] + S[3]
            halve(aview(1), view(w8t, S[0] * 8, S[1], 8), 16)
            halve(aview(2), view(s0t, 0, S[2], 16), 32)
            halve(view(s0t, 0, S[2], 16),
                  view(w8t, (S[0] + S[1]) * 8, S[2], 8), 16)
            halve(aview(3), view(s0t, 0, S[3], 32), 64)
            halve(view(s0t, 0, S[3], 32), view(s1t, 0, S[3], 16), 32)
            halve(view(s1t, 0, S[3], 16),
                  view(w8t, (S[0] + S[1] + S[2]) * 8, S[3], 8), 16)
            # shared 8 -> 4 -> 2 -> 1
            halve(view(w8t, 0, Sp, 8), view(s0t, 0, Sp, 4), 8)
            halve(view(s0t, 0, Sp, 4), view(s1t, 0, Sp, 2), 4)
            nc.vector.tensor_tensor(
                out=Pb[:, S01:S01 + Sp].rearrange("p (s o) -> p s o", o=1),
                in0=view(s1t, 0, Sp, 2)[:, :, 0:1],
                in1=view(s1t, 0, Sp, 2)[:, :, 1:2], op=OP.mult)

            # ---- early epilogue ACT part (needs only BHE); hv/uv are
            # host-masked to 1.0 for censored rows, so the plain Ln accums
            # give the e-weighted sums directly
            nc.scalar.activation(Lb[:], BHE[:, 0:2 * P], AF.Ln,
                                 accum_out=acc[:, 2:3])
            nc.scalar.activation(loghv[:], BHE[:, 2 * P:3 * P], AF.Ln,
                                 accum_out=acc[:, 3:4])
            nc.scalar.activation(sct[:], BHE[:, 3 * P:4 * P], AF.Ln,
                                 accum_out=acc[:, 6:7])

            def epilogue_half(h, lo, hi):
                hs = slice(lo, hi)
                nc.scalar.activation(Ab[:, hs], Pb[:, hs], AF.Ln,
                                     accum_out=acc[:, 0 + h:1 + h])
                # C2 is host-masked by e, so Pt = e * P: censored rows get
                # wt = 1 -> ln(wt + 1e-8) ~ 0 and the Ln accum IS T_ewt
                nc.vector.tensor_tensor(out=Pt[:, hs], in0=Pb[:, hs],
                                        in1=C2t[:, hs], op=OP.mult)
                # v>=2 rows always have 1-P >= 0.0078 (h >= 0.01), and
                # v01/censored rows have Pt = 0 (C2-masked) -> ln(1) = 0;
                # their true ln(1e-8) rides a host-side count constant
                nc.scalar.activation(logwt[:, hs], Pt[:, hs], AF.Ln,
                                     bias=1.0, scale=-1.0,
                                     accum_out=acc[:, 7 + h:8 + h])
                nc.vector.scalar_tensor_tensor(
                    out=loghv[:, hs], in0=Ab[:, hs], scalar=0.0,
                    in1=Eb[:, hs],
                    op0=OP.add, op1=OP.mult, accum_out=acc[:, 4 + h:5 + h])

            epilogue_half(0, 0, S01 + Sp)

            # ---- classes 24, 48 feed shared width-12 buffer ----
            halve(aview(4), view(w12t, 0, S[4], 12), 24)
            halve(aview(5), view(s0t, 0, S[5], 24), 48)
            halve(view(s0t, 0, S[5], 24),
                  view(w12t, S[4] * 12, S[5], 12), 24)
            S45 = S[4] + S[5]
            halve(view(w12t, 0, S45, 12), view(s0t, 0, S45, 6), 12)
            halve(view(s0t, 0, S45, 6), view(s1t, 0, S45, 3), 6)
            nc.vector.tensor_reduce(Pb[:, pcol[4]:pcol[6]],
                                    view(s1t, 0, S45, 3), axis=AX.X,
                                    op=OP.mult)

            # ---- classes 40, 56 ----
            halve(aview(6), view(s0t, 0, S[6], 20), 40)
            halve(view(s0t, 0, S[6], 20), view(s1t, 0, S[6], 10), 20)
            halve(view(s1t, 0, S[6], 10), view(s0t, 0, S[6], 5), 10)
            nc.vector.tensor_reduce(Pb[:, pcol[6]:pcol[7]],
                                    view(s0t, 0, S[6], 5), axis=AX.X,
                                    op=OP.mult)
            halve(aview(7), view(s0t, 0, S[7], 28), 56)
            halve(view(s0t, 0, S[7], 28), view(s1t, 0, S[7], 14), 28)
            halve(view(s1t, 0, S[7], 14), view(s0t, 0, S[7], 7), 14)
            nc.vector.tensor_reduce(Pb[:, pcol[7]:pcol[8]],
                                    view(s0t, 0, S[7], 7), axis=AX.X,
                                    op=OP.mult)

            epilogue_half(1, S01 + Sp, P)

            nc.sync.dma_start(part_d[:], acc[:])

    nc.finalize()
    return nc


def _pack_core(preds_rows, ev_rows, idx_rows, plan):
    """Pack one core's rows -> abuf [128, AW], bhe [128, 4P], c2 [128, P],
    corrA.  Class-contiguous: rows of a class (sorted by v) fill slots
    r -> (partition r // S_c, col r % S_c)."""
    S01, S, pcol, P, aoff, AW = (plan[k] for k in
                                 ("S01", "S", "pcol", "P", "aoff", "AW"))
    u = (1.0 - preds_rows).astype(BF16)
    u2 = (2.0 - 2.0 * preds_rows).astype(BF16)

    abuf = np.ones((128, AW), BF16)
    bhe = np.empty((128, 5 * P), BF16)
    c2 = np.ones((128, P), BF16)
    bhe[:, 0:4 * P] = BF16(1.0)
    bhe[:, 4 * P:5 * P] = BF16(0.0)

    order = np.argsort(idx_rows, kind="stable")
    counts = np.bincount(idx_rows, minlength=G)
    starts = np.concatenate([[0], np.cumsum(counts)])
    corrA = 0.0

    def fill_block(rows, vvec, col0, Sc, pad_exp):
        n = len(rows)
        cap = 128 * Sc
        blk = np.empty((5, cap), BF16)
        blk[0:4] = BF16(1.0)
        blk[4] = BF16(0.0)
        vm1 = np.maximum(vvec - 1, 0)
        em = ev_rows[rows] > 0.5
        blk[0, :n] = np.where(vvec >= 1, u[rows, vm1], BF16(1.0))
        blk[1, :n] = u[rows, vvec]
        blk[2, :n] = np.where(em, preds_rows[rows, vvec].astype(BF16),
                              BF16(1.0))
        blk[3, :n] = np.where(em, u[rows, vvec], BF16(1.0))
        blk[4, :n] = ev_rows[rows].astype(BF16)
        for t in range(5):
            bhe[:, t * P + col0:t * P + col0 + Sc] = blk[t].reshape(128, Sc)
        ex = np.full(cap, float(pad_exp))
        ex[:n] = vm1
        c2v = np.zeros(cap)
        c2v[:n] = np.where(em, 2.0 ** (-vm1.astype(np.float64)), 0.0)
        c2[:, col0:col0 + Sc] = c2v.astype(BF16).reshape(128, Sc)
        return ex

    rows01 = order[starts[0]:starts[2]]
    fill_block(rows01, idx_rows[rows01], 0, S01, 0)
    c2[:, 0:S01] = BF16(0.0)     # v01: logwt constant handled on host

    for ci, (w, v0, v1) in enumerate(CLASSES):
        rows = order[starts[v0]:starts[v1 + 1]]
        vvec = idx_rows[rows]
        n = len(rows)
        Sc = S[ci]
        cap = 128 * Sc
        assert n <= cap, f"class {ci} overflow: {n} > {cap}"
        ex = fill_block(rows, vvec, pcol[ci], Sc, w)
        corrA += LN2 * float(ex.sum())
        ab = np.full((cap, w), BF16(2.0))
        colmask = np.arange(w)[None, :] < (vvec - 1)[:, None]
        ab[:n] = np.where(colmask, u2[rows][:, :w], BF16(1.0))
        abuf[:, aoff[ci]:aoff[ci] + Sc * w] = ab.reshape(128, Sc * w)
    return abuf, bhe, c2, corrA


def _combine(partials_list, b_total, corr_eA, corrA_total, sum_e,
             corr_wt01):
    s = np.zeros(12, np.float64)
    for p in partials_list:
        s += p.astype(np.float64).sum(axis=0)
    T_A = s[0] + s[1] - corrA_total
    T_LB, T_lh = s[2], s[3]
    T_eA = s[4] + s[5] - corr_eA
    T_elgv = s[6]
    T_ewt = s[7] + s[8] + corr_wt01
    L_z = -(T_lh + T_eA) / sum_e
    L_c = -(T_A - T_eA + T_ewt) / b_total
    nll = -((T_A + T_LB) + (T_lh - T_elgv)) / b_total
    return np.float32(0.5 * L_z + 0.5 * L_c + nll)


def kernel(preds: np.ndarray, target: np.ndarray) -> np.ndarray:
    from concourse.bass_utils import run_bass_kernel_spmd

    preds = np.asarray(preds, np.float32).reshape(B_TOTAL, T)
    target = np.asarray(target, np.float32).reshape(B_TOTAL, 3)
    idx = target[:, 0].astype(np.int64)
    ev = target[:, 1].astype(np.float64)

    core = np.arange(B_TOTAL) % N_CORES
    all_counts = np.stack([np.bincount(idx[core == c], minlength=G)
                           for c in range(N_CORES)])
    plan = _plan(all_counts)
    key = (plan["S01"],) + tuple(plan["S"])
    if _CACHE.get("key") != key:
        _CACHE["nc"] = _build_nc(plan)
        _CACHE["key"] = key
    nc = _CACHE["nc"]

    corr_eA = LN2 * float(np.sum(ev * np.maximum(idx - 1, 0)))
    sum_e = float(ev.sum())
    corr_wt01 = float(np.log(1e-8)) * float(ev[idx <= 1].sum())
    chunks = _chunks(plan)
    in_maps = []
    corrA_total = 0.0
    for c in range(N_CORES):
        m = core == c
        abuf, bhe, c2, corrA = _pack_core(preds[m], ev[m].astype(np.float32),
                                          idx[m], plan)
        corrA_total += corrA
        im = {"bhe": bhe, "c2": c2}
        for i, (c0, c1) in chunks.items():
            im[f"ab{i}"] = np.ascontiguousarray(abuf[:, c0:c1])
        in_maps.append(im)

    res = run_bass_kernel_spmd(nc, in_maps, core_ids=list(range(N_CORES)))
    _CACHE["last_results"] = res
    return _combine([r["partials"] for r in res.results], float(B_TOTAL),
                    corr_eA, corrA_total, sum_e, corr_wt01)


if __name__ == "__main__":
    pass

